# revision 12
# baseline (speedup 1.0000x reference)
"""CombinedLoss (0.8*Dice + 0.2*masked-MSE) on 8 Trainium2 NeuronCores.

Strategy
--------
Inputs are [16,3,512,512] f32 logits/targets (targets binary {0,1}).
Everything the loss needs collapses to 6 scalar sums per (b,c) pair:

    P  = sum(sigmoid(l))          A  = sum(sigmoid(l)*t)
    L2 = sum(l^2)                 LT = sum(l*t)
    U  = sum(t*l^2) = sum((l*t)^2)  (t binary)
    T  = sum(t)     = sum(t*t)      (t binary)

from which:  inter=A, union=P+T, sse_pos=U-2*LT+T, sse_neg=L2-U,
             cnt_pos=T, cnt_neg=H*W-T.

Data-parallel over batch: core k handles batches [2k, 2k+1] -> 6 (b,c)
pairs per core, each pair viewed as a [128, 2048] tile.  Host casts
inputs to bf16 (targets exact; logits add ~1e-5 statistical error to the
262144-element sums) which halves DMA bytes and unlocks DVE 2x mode.

Per pair on device:
  ACT: sigmoid(l) -> p (bf16) with accum_out P ;  Square(l) with accum_out L2
  DVE: tensor_tensor_reduce fused mul+sum for LT (materializes lt), A,
       U (=sum(lt*lt)), T (=sum(t*t))
All partials are [128,1] f32 columns of per-quantity [128,6] tiles; one
[128,36] f32 output per core; host does the final 128-way sums and the
scalar combine.
"""

from contextlib import ExitStack

import numpy as np
import ml_dtypes

import concourse.bass as bass
import concourse.tile as tile
import concourse.mybir as mybir
from concourse.bass_utils import run_bass_kernel_spmd

BF16 = ml_dtypes.bfloat16


def _patch_sem_clear():
    """The walrus build in this container rejects the
    EVENT_SEMAPHORE_RANGE_CLEAR ISA op ("ISA wrong length") that Tile's
    semaphore-release path emits via gpsimd.sem_clear.  Reset the range with
    the (supported) drain-with-semaphore-range instruction instead."""
    if getattr(bass.Bass, "_sem_clear_patched", False):
        return

    def clear_and_free_semaphores(self, sems):
        if not sems:
            return
        sem_nums = [
            s.num if isinstance(s, bass.SemaphoreHandle) else s for s in sems
        ]
        for sem_range in bass.compact_to_ranges(sem_nums):
            assert self._state.free_isdisjoint(sem_range)
            self.gpsimd.dma_reset(sem_range)
            self.gpsimd.drain(semaphore_range=sem_range)
        self._state.prepend_free_semaphores(sem_nums)
        for poison_set in self._tile_sem_poison_stack:
            poison_set.update(sem_nums)

    bass.Bass.clear_and_free_semaphores = clear_and_free_semaphores
    bass.Bass._sem_clear_patched = True


def _patch_single_dma_sem():
    """This walrus also allows only ONE sync-wait per compute instruction.
    Tile spreads HWDGE DMA completions over 8 semaphore lanes, so an op
    waiting on two DMAs gets two waits.  Funnel all HWDGE DMAs through one
    lane (sound: qSPDynamicHW is FIFO per issuing engine) so Tile merges
    such waits into a single max-value wait."""
    import concourse.tile_sem_assignment as tsa

    tsa.NUM_HWDGE_SEMS = 1


_patch_sem_clear()
_patch_single_dma_sem()


def _legalize_waits(nc) -> None:
    """This walrus accepts at most one sync-wait per instruction.  Split any
    instruction carrying N>1 waits into N-1 preceding wait-only drains on the
    same engine (waits AND together, so order is irrelevant)."""
    for fn in nc.m.functions:
        for blk in fn.blocks:
            insts = blk.instructions
            out = []
            changed = False
            for inst in insts:
                si = getattr(inst, "sync_info", None)
                waits = list(si.on_wait) if si and si.on_wait else []
                if len(waits) > 1:
                    changed = True
                    for w in waits[:-1]:
                        d = mybir.InstDrain(
                            name=f"{inst.name}_w{w.id}",
                            ins=[],
                            outs=[],
                            bass_is_fusable=False,
                        )
                        d.engine = inst.engine
                        d.sync_info = mybir.SyncInfo(on_wait=[w], on_update=[])
                        out.append(d)
                    inst.sync_info = mybir.SyncInfo(
                        on_wait=[waits[-1]], on_update=list(si.on_update or [])
                    )
                out.append(inst)
            if changed:
                blk.instructions = out

B, C, H, W = 16, 3, 512, 512
N_CORES = 8
B_LOC = B // N_CORES            # 2 batches per core
PAIRS = B_LOC * C               # 6 (b,c) pairs per core
P = 128                         # SBUF partitions
F = (H * W) // P                # 2048 free elements per partition
HW = float(H * W)

SMOOTH = 0.001
DICE_WEIGHT = 0.8

# quantity order for PE-reduced sums: psum row r = 4*pair + q
QPE_T, QPE_LT, QPE_A, QPE_U = range(4)
NQPE = 4
NROWS = NQPE * PAIRS          # 24 psum rows
NCHUNK = 4                    # F split into 512-wide matmul chunks
CHUNK = F // NCHUNK           # 512 (one PSUM bank)

# ACT-accumulated quantities: acc column = q*PAIRS + pair
QA_P, QA_L2 = range(2)
NQA = 2


def _build_nc() -> bass.Bass:
    nc = bass.Bass()
    l_in = nc.dram_tensor(
        "logits", [PAIRS, P, F], mybir.dt.bfloat16, kind="ExternalInput"
    )
    t_in = nc.dram_tensor(
        "targets", [PAIRS, P, F], mybir.dt.bfloat16, kind="ExternalInput"
    )
    # one-hot ones column at index NROWS-1 of [P, 2*NROWS-1]
    w_in = nc.dram_tensor(
        "w", [P, 2 * NROWS - 1], mybir.dt.bfloat16, kind="ExternalInput"
    )
    acc_out = nc.dram_tensor(
        "acc", [P, NQA * PAIRS], mybir.dt.float32, kind="ExternalOutput"
    )
    sums_out = nc.dram_tensor(
        "sums", [NROWS, 1], mybir.dt.float32, kind="ExternalOutput"
    )

    FT = mybir.ActivationFunctionType
    OP = mybir.AluOpType
    f32 = mybir.dt.float32
    bf16 = mybir.dt.bfloat16

    with tile.TileContext(nc) as tc, ExitStack() as ctx:
        inp = ctx.enter_context(tc.tile_pool(name="inp", bufs=PAIRS))
        mid = ctx.enter_context(tc.tile_pool(name="mid", bufs=PAIRS))
        scr = ctx.enter_context(tc.tile_pool(name="scr", bufs=PAIRS))
        accs = ctx.enter_context(tc.tile_pool(name="accs", bufs=1))
        psums = ctx.enter_context(
            tc.tile_pool(name="psums", bufs=1, space="PSUM")
        )

        wt = accs.tile([P, 2 * NROWS - 1], bf16, name="wt")
        nc.sync.dma_start(out=wt[:], in_=w_in[:, :])

        # ACT accumulators (P, L2) — written only by ACT
        acc_a = accs.tile([P, NQA * PAIRS], f32, name="acc_a")
        # PE accumulation bank: rows = 4*pair + q, cols = chunk position
        ps = psums.tile([NROWS, CHUNK], f32, name="ps")

        mm_idx = [0]

        def pe_reduce(rhs_tile, pair, q):
            r = NQPE * pair + q
            lhsT = wt[:, NROWS - 1 - r : 2 * NROWS - 1 - r]
            for c in range(NCHUNK):
                nc.tensor.matmul(
                    ps[:, :],
                    lhsT,
                    rhs_tile[:, c * CHUNK : (c + 1) * CHUNK],
                    start=(mm_idx[0] == 0),
                    stop=(mm_idx[0] == PAIRS * NQPE * NCHUNK - 1),
                )
                mm_idx[0] += 1

        for i in range(PAIRS):
            lt_i = inp.tile([P, F], bf16, tag="l", name=f"l{i}")
            tt_i = inp.tile([P, F], bf16, tag="t", name=f"t{i}")
            nc.sync.dma_start(out=lt_i[:], in_=l_in[i, :, :])
            nc.sync.dma_start(out=tt_i[:], in_=t_in[i, :, :])

            # T = sum(t) on PE (t is ready first)
            pe_reduce(tt_i, i, QPE_T)

            # ACT: p = sigmoid(l), accumulating P
            p_i = mid.tile([P, F], bf16, tag="p", name=f"p{i}")
            nc.scalar.activation(
                out=p_i[:],
                in_=lt_i[:],
                func=FT.Sigmoid,
                accum_out=acc_a[:, QA_P * PAIRS + i : QA_P * PAIRS + i + 1],
            )
            # ACT: square(l), accumulating L2
            asq_i = scr.tile([P, F], bf16, tag="asq", name=f"asq{i}")
            nc.scalar.activation(
                out=asq_i[:],
                in_=lt_i[:],
                func=FT.Square,
                accum_out=acc_a[:, QA_L2 * PAIRS + i : QA_L2 * PAIRS + i + 1],
            )

            # DVE: lt = l*t ; PE reduces -> LT
            ltp_i = mid.tile([P, F], bf16, tag="ltp", name=f"ltp{i}")
            nc.vector.tensor_mul(ltp_i[:], lt_i[:], tt_i[:])
            pe_reduce(ltp_i, i, QPE_LT)

            # DVE: pt = p*t ; PE reduces -> A
            pts_i = scr.tile([P, F], bf16, tag="pts", name=f"pts{i}")
            nc.vector.tensor_mul(pts_i[:], p_i[:], tt_i[:])
            pe_reduce(pts_i, i, QPE_A)

            # DVE: (l*t)^2 ; PE reduces -> U
            usq_i = scr.tile([P, F], bf16, tag="usq", name=f"usq{i}")
            nc.vector.tensor_mul(usq_i[:], ltp_i[:], ltp_i[:])
            pe_reduce(usq_i, i, QPE_U)

        # collapse psum free dim -> [NROWS, 1] and ship everything out
        sums_t = accs.tile([NROWS, 1], f32, name="sums_t")
        nc.vector.tensor_reduce(
            out=sums_t[:], in_=ps[:, :], axis=mybir.AxisListType.X, op=OP.add
        )
        # SWDGE for outputs: separate DMASW lanes keep each at one sync-wait
        nc.gpsimd.dma_start(out=sums_out[:, :], in_=sums_t[:])
        nc.gpsimd.dma_start(out=acc_out[:, :], in_=acc_a[:])

    _legalize_waits(nc)
    return nc


_NC_CACHE = None


def _get_nc() -> bass.Bass:
    global _NC_CACHE
    if _NC_CACHE is None:
        _NC_CACHE = _build_nc()
    return _NC_CACHE


def _shard(x: np.ndarray) -> np.ndarray:
    """[B,C,H,W] f32 -> [N_CORES, PAIRS, P, F] bf16 (batch-sharded)."""
    x = np.asarray(x, dtype=np.float32).reshape(N_CORES, PAIRS, P, F)
    return x.astype(BF16)


def _make_w() -> np.ndarray:
    w = np.zeros((P, 2 * NROWS - 1), dtype=BF16)
    w[:, NROWS - 1] = 1
    return w


def _combine(results: list[dict]) -> np.float32:
    # acc: [P, NQA*PAIRS] f32 ACT partials; sums: [NROWS,1] PE sums
    Sa = np.stack(
        [r["acc"].astype(np.float64).sum(axis=0) for r in results]
    ).reshape(N_CORES, NQA, PAIRS)
    Sp = np.stack([r["sums"].astype(np.float64)[:, 0] for r in results]).reshape(
        N_CORES, PAIRS, NQPE
    )

    def bc_a(q):  # -> [B, C]
        return Sa[:, q, :].reshape(B, C)

    def bc_p(q):
        return Sp[:, :, q].reshape(B, C)

    P_, L2 = bc_a(QA_P), bc_a(QA_L2)
    T, LT, A, U = bc_p(QPE_T), bc_p(QPE_LT), bc_p(QPE_A), bc_p(QPE_U)

    inter = A
    union = P_ + T
    dice = (2.0 * inter + SMOOTH) / (union + SMOOTH)
    dice_loss = np.sum(1.0 - dice.mean(axis=0)) / C

    cnt_pos = T
    cnt_neg = HW - T
    sse_pos = U - 2.0 * LT + T
    sse_neg = L2 - U
    mse_pos = np.where(cnt_pos > 0, sse_pos / np.maximum(cnt_pos, 1.0), 0.0)
    mse_neg = np.where(cnt_neg > 0, sse_neg / np.maximum(cnt_neg, 1.0), 0.0)
    mse_loss = np.sum(mse_pos + mse_neg) / C / B

    return np.float32(DICE_WEIGHT * dice_loss + (1.0 - DICE_WEIGHT) * mse_loss)


def kernel(logits, targets, _trace=False, _return_results=False):
    lsh = _shard(logits)
    tsh = _shard(targets)
    w = _make_w()
    in_maps = [
        {"logits": lsh[k], "targets": tsh[k], "w": w} for k in range(N_CORES)
    ]
    nc = _get_nc()
    try:
        res = run_bass_kernel_spmd(
            nc, in_maps, core_ids=list(range(N_CORES)), trace=_trace
        )
    except ModuleNotFoundError:
        # axon NTFF profile hook unavailable in this container
        res = run_bass_kernel_spmd(
            nc, in_maps, core_ids=list(range(N_CORES)), trace=False
        )
    out = _combine(res.results)
    if _return_results:
        return out, res
    return out
